# revision 4
# baseline (speedup 1.0000x reference)
"""Trainium2 Bass kernel v3 — fp8 weights + software-pipelined schedule.

Differences vs v2:
- Reordered steady-state iteration so the PE queue never head-blocks:
  A=pre1wh(t), C=pre1wi(t), D=pre0h(t+1), tg1 transposes, B=pre0x(t+2)
  (PE filler during the gate0/AG window), tg0 transposes. The layer-0
  pipeline runs one step ahead of layer 1, so both AllGathers launch evenly
  spaced and their lands complete before the consuming matmuls.
- h lands split into 4 chunked DMAs (2 k-tiles each) so pre1wh can start
  ~1us after the collective completes.
- ps0 double-buffered (pre0x for step t+2 is issued while gate0 of step
  t+1 still reads the previous buffer).
"""
import numpy as np

B, I, R, L, U = 256, 512, 1024, 2, 11
U_RUN = U - 1
N_CORES = 8
RC = 128
W = 4 * RC
NKX = I // 128         # 4
NKH = R // 128         # 8
NK0 = NKX + NKH        # 12
NK1 = 2 * NKH          # 16
NB = B // 128
BH = 128
WSCALE = 64.0


def build_program(reps: int = 1):
    import concourse.bacc as bacc
    import concourse.mybir as mybir
    import concourse.tile as tile
    import concourse.masks as masks

    F32 = mybir.dt.float32
    BF16 = mybir.dt.bfloat16
    FP8 = mybir.dt.float8e4
    Sig = mybir.ActivationFunctionType.Sigmoid
    Tanh = mybir.ActivationFunctionType.Tanh
    INV = 1.0 / WSCALE

    PHASE_MARKS.clear()
    nc = bacc.Bacc("TRN2", target_bir_lowering=False, debug=False,
                   num_devices=N_CORES)

    xT_d = nc.dram_tensor("xT", [I, B], BF16, kind="ExternalInput")
    h0T_d = nc.dram_tensor("h0T0", [R, B], BF16, kind="ExternalInput")
    h1T_d = nc.dram_tensor("h1T0", [R, B], BF16, kind="ExternalInput")
    c0_d = nc.dram_tensor("c0i", [B, RC], F32, kind="ExternalInput")
    c1_d = nc.dram_tensor("c1i", [B, RC], F32, kind="ExternalInput")
    wall_d = nc.dram_tensor("wall", [U_RUN, 128, (NK0 + NK1) * W], FP8,
                            kind="ExternalInput")
    b0_d = nc.dram_tensor("b0", [1, U_RUN * W], BF16, kind="ExternalInput")
    b1_d = nc.dram_tensor("b1", [1, U_RUN * W], BF16, kind="ExternalInput")
    y_d = nc.dram_tensor("y", [U_RUN, B, RC], BF16, kind="ExternalOutput")

    with tile.TileContext(nc) as tc:
        with tc.tile_pool(name="const", bufs=1) as constp, \
             tc.tile_pool(name="hpool", bufs=2) as hpool, \
             tc.tile_pool(name="cpool", bufs=2) as cpool, \
             tc.tile_pool(name="wpool", bufs=3) as wpool, \
             tc.tile_pool(name="gpool", bufs=2) as gpool, \
             tc.tile_pool(name="stage", bufs=2) as stage, \
             tc.tile_pool(name="ps0", bufs=2, space="PSUM") as ps0, \
             tc.tile_pool(name="ps1", bufs=1, space="PSUM") as ps1, \
             tc.tile_pool(name="psT", bufs=2, space="PSUM") as psT, \
             tc.tile_pool(name="dramp", bufs=2, space="DRAM") as dramp:

            ident = constp.tile([128, 128], BF16)
            masks.make_identity(nc, ident[:])
            ones = constp.tile([1, 128], BF16)
            nc.gpsimd.memset(ones[:], 1.0)
            xT_tiles = []
            for kk in range(NKX):
                xt = constp.tile([128, B], BF16, name=f"xT_{kk}")
                nc.sync.dma_start(xt[:], xT_d.ap()[kk * 128 : (kk + 1) * 128, :])
                xT_tiles.append(xt)
            bias0 = constp.tile([1, U_RUN * W], BF16)
            nc.sync.dma_start(bias0[:], b0_d.ap())
            bias1 = constp.tile([1, U_RUN * W], BF16)
            nc.sync.dma_start(bias1[:], b1_d.ap())

            def load_h_half(layer, b, src_ap, chunks=2):
                """[R, BH] h^T half -> [128, NKH*BH] tile."""
                t = hpool.tile([128, NKH * BH], BF16, name=f"h{layer}T_{b}",
                               tag=f"h{layer}T_{b}")
                step = NKH // chunks
                for ci in range(chunks):
                    lo = ci * step
                    nc.sync.dma_start(
                        t[:, lo * BH : (lo + step) * BH]
                        .rearrange("p (k b) -> p k b", k=step),
                        src_ap[lo * 128 : (lo + step) * 128, :]
                        .rearrange("(k p) b -> p k b", p=128),
                    )
                return t

            def load_wall(src_ap):
                ta = wpool.tile([128, NK0 * W], FP8, name="w0_s", tag="w0_s")
                for lo in range(0, NK0, 4):
                    hi = min(lo + 4, NK0)
                    nc.scalar.dma_start(ta[:, lo * W : hi * W],
                                        src_ap[:, lo * W : hi * W])
                tb = wpool.tile([128, NK1 * W], FP8, name="w1_s", tag="w1_s")
                for lo in range(0, NK1, 4):
                    hi = min(lo + 4, NK1)
                    nc.scalar.dma_start(tb[:, lo * W : hi * W],
                                        src_ap[:, (NK0 + lo) * W : (NK0 + hi) * W])
                return ta, tb

            def gate_cell(layer, b, pre_ps, c_old, hn_tile):
                sig = gpool.tile([128, 3 * RC], F32, name=f"sig{layer}_{b}",
                                 tag=f"sig{layer}_{b}")
                nc.scalar.activation(sig[:], pre_ps[:, : 3 * RC], Sig, scale=INV)
                tg = gpool.tile([128, RC], F32, name=f"tg{layer}_{b}",
                                tag=f"tg{layer}_{b}")
                nc.scalar.activation(tg[:], pre_ps[:, 3 * RC :], Tanh, scale=INV)
                t1 = gpool.tile([128, RC], F32, name=f"t1{layer}_{b}",
                                tag=f"t1{layer}_{b}")
                nc.vector.tensor_mul(t1[:], sig[:, :RC], tg[:])
                t2 = gpool.tile([128, RC], F32, name=f"t2{layer}_{b}",
                                tag=f"t2{layer}_{b}")
                nc.vector.tensor_mul(t2[:], sig[:, RC : 2 * RC], c_old[:])
                cnew = cpool.tile([128, RC], F32, name=f"c{layer}_{b}",
                                  tag=f"c{layer}_{b}")
                nc.vector.tensor_add(cnew[:], t1[:], t2[:])
                tcc = gpool.tile([128, RC], F32, name=f"tc{layer}_{b}",
                                 tag=f"tc{layer}_{b}")
                nc.scalar.activation(tcc[:], cnew[:], Tanh)
                nc.vector.tensor_mul(hn_tile[:, b * RC : (b + 1) * RC],
                                     sig[:, 2 * RC : 3 * RC], tcc[:])
                return cnew

            def gather_half(layer, b, hn_tile):
                """Transpose + cast + stage + AllGather for one half."""
                tps = psT.tile([128, BH], BF16, name=f"tps{layer}{b}", tag="tps")
                nc.tensor.transpose(tps[:], hn_tile[:, b * RC : (b + 1) * RC],
                                    ident[:])
                hst = stage.tile([128, BH], BF16, name=f"hst{layer}_{b}",
                                 tag=f"hst{layer}_{b}")
                nc.vector.tensor_copy(hst[:], tps[:])
                ag_in = dramp.tile([128, BH], BF16, name=f"agin{layer}_{b}",
                                   tag=f"agin{layer}_{b}")
                nc.sync.dma_start(ag_in[:], hst[:])
                ag_out = dramp.tile([R, BH], BF16, name=f"agout{layer}_{b}",
                                    tag=f"agout{layer}_{b}",
                                    addr_space="Shared")
                nc.gpsimd.collective_compute(
                    "AllGather", mybir.AluOpType.bypass,
                    replica_groups=[list(range(N_CORES))],
                    ins=[ag_in[:]], outs=[ag_out[:]],
                )
                return ag_out

            def hs(h_t, kk):
                return h_t[:, kk * BH : (kk + 1) * BH]

            def mm(p, lhsT, wslice, start, stop):
                nc.tensor.matmul(p[:], lhsT, wslice, start=start, stop=stop)

            def pre0_start(widx, wall_t, b):
                """bias + x part for step widx (opens the ps0 group)."""
                w0_s, _ = wall_t
                p = ps0.tile([128, W], F32, name=f"pre0_{b}", tag=f"pre0_{b}")
                mm(p, ones[:], bias0[:, widx * W : (widx + 1) * W], True, False)
                for kk in range(NKX):
                    mm(p, xT_tiles[kk][:, b * BH : (b + 1) * BH],
                       w0_s[:, kk * W : (kk + 1) * W], False, False)
                return p

            def pre0_finish(p, h0T_b, wall_t):
                w0_s, _ = wall_t
                for kh in range(NKH):
                    mm(p, hs(h0T_b, kh),
                       w0_s[:, (NKX + kh) * W : (NKX + kh + 1) * W],
                       False, kh == NKH - 1)

            def pre1_wh(widx, wall_t, h1T_b, b):
                _, w1_s = wall_t
                p = ps1.tile([128, W], F32, name=f"pre1_{b}", tag=f"pre1_{b}")
                mm(p, ones[:], bias1[:, widx * W : (widx + 1) * W], True, False)
                for kh in range(NKH):
                    mm(p, hs(h1T_b, kh),
                       w1_s[:, kh * W : (kh + 1) * W], False, False)
                return p

            def pre1_wi(p, h0T_b, wall_t):
                _, w1_s = wall_t
                for kh in range(NKH):
                    mm(p, hs(h0T_b, kh),
                       w1_s[:, (NKH + kh) * W : (NKH + kh + 1) * W],
                       False, kh == NKH - 1)

            # ---- prologue ----
            h0T = {b: load_h_half(0, b, h0T_d.ap()[:, b * BH:(b + 1) * BH])
                   for b in range(NB)}
            h1T = {b: load_h_half(1, b, h1T_d.ap()[:, b * BH:(b + 1) * BH])
                   for b in range(NB)}
            c = {}
            for layer, cd in ((0, c0_d), (1, c1_d)):
                for b in range(NB):
                    t = cpool.tile([128, RC], F32, name=f"c{layer}_{b}",
                                   tag=f"c{layer}_{b}")
                    nc.sync.dma_start(t[:], cd.ap()[b * 128 : (b + 1) * 128, :])
                    c[layer, b] = t

            wall = {0: load_wall(wall_d.ap()[0]), 1: load_wall(wall_d.ap()[1])}
            # step-0 layer0 fully in prologue
            pre0_pend = [pre0_start(0, wall[0], b) for b in range(NB)]
            for b in range(NB):
                pre0_finish(pre0_pend[b], h0T[b], wall[0])
            h0new = gpool.tile([128, NB * RC], BF16, name="h0new", tag="h0new")
            for b in range(NB):
                c[0, b] = gate_cell(0, b, pre0_pend[b], c[0, b], h0new)
                ag = gather_half(0, b, h0new)
                h0T[b] = load_h_half(0, b, ag[:])
            # open step-1 layer0 group (bias + x part) as PE filler
            pre0_pend = [pre0_start(1, wall[1], b) for b in range(NB)]

            # ---- steady-state iterations ----
            for rep in range(reps):
                for t in range(U_RUN):
                    has_next = not (rep == reps - 1 and t == U_RUN - 1)
                    write_y = rep == 0
                    nidx = (t + 1) % U_RUN
                    n2idx = (t + 2) % U_RUN

                    if has_next:
                        _mark(nc, f"r{rep}t{t}:wdma")
                        wall[n2idx] = load_wall(wall_d.ap()[n2idx])

                    _mark(nc, f"r{rep}t{t}:pre1")
                    wt = wall[t % U_RUN]
                    pre1 = []
                    for b in range(NB):
                        p = pre1_wh(t, wt, h1T[b], b)
                        pre1_wi(p, h0T[b], wt)
                        pre1.append(p)

                    if has_next:
                        _mark(nc, f"r{rep}t{t}:pre0h")
                        for b in range(NB):
                            pre0_finish(pre0_pend[b], h0T[b], wall[nidx])

                    _mark(nc, f"r{rep}t{t}:gate1")
                    h1new = gpool.tile([128, NB * RC], BF16, name="h1new",
                                       tag="h1new")
                    ag1 = {}
                    for b in range(NB):
                        c[1, b] = gate_cell(1, b, pre1[b], c[1, b], h1new)
                        if has_next:
                            ag1[b] = gather_half(1, b, h1new)
                    if write_y:
                        nc.sync.dma_start(
                            y_d.ap()[t].rearrange("(nb p) r -> p nb r", p=128),
                            h1new[:].rearrange("p (nb r) -> p nb r", nb=NB))

                    if has_next:
                        _mark(nc, f"r{rep}t{t}:gate0")
                        h0new = gpool.tile([128, NB * RC], BF16, name="h0new",
                                           tag="h0new")
                        ag0 = {}
                        for b in range(NB):
                            c[0, b] = gate_cell(0, b, pre0_pend[b], c[0, b],
                                                h0new)
                            ag0[b] = gather_half(0, b, h0new)

                        has_n2 = not (rep == reps - 1 and t == U_RUN - 2)
                        if has_n2:
                            _mark(nc, f"r{rep}t{t}:xpart")
                            pre0_pend = [pre0_start(n2idx, wall[n2idx], b)
                                         for b in range(NB)]

                        _mark(nc, f"r{rep}t{t}:lands")
                        for b in range(NB):
                            h1T[b] = load_h_half(1, b, ag1[b][:])
                        for b in range(NB):
                            h0T[b] = load_h_half(0, b, ag0[b][:])

    _mark(nc, "end")
    nc.compile()
    return nc


def prepare_in_maps(inputs: dict) -> list[dict]:
    import ml_dtypes
    bf = ml_dtypes.bfloat16
    fp8 = ml_dtypes.float8_e4m3

    x = np.ascontiguousarray(np.asarray(inputs["x"], np.float32))
    st = np.asarray(inputs["init_states_input"], np.float32).reshape(B, 2 * L, R)
    h0i, c0i, h1i, c1i = st[:, 0], st[:, 1], st[:, 2], st[:, 3]

    xT = x.T.astype(bf)
    h0T = h0i.T.astype(bf)
    h1T = h1i.T.astype(bf)

    Wi0 = np.asarray(inputs["Wi0"], np.float32)[:U_RUN] * WSCALE
    Wh0 = np.asarray(inputs["Wh0"], np.float32)[:U_RUN] * WSCALE
    Wi1 = np.asarray(inputs["Wi1"], np.float32)[:U_RUN] * WSCALE
    Wh1 = np.asarray(inputs["Wh1"], np.float32)[:U_RUN] * WSCALE
    b0_full = (np.asarray(inputs["bi0"], np.float32)
               + np.asarray(inputs["bh0"], np.float32))[:U_RUN] * WSCALE
    b1_full = (np.asarray(inputs["bi1"], np.float32)
               + np.asarray(inputs["bh1"], np.float32))[:U_RUN] * WSCALE

    in_maps = []
    for k in range(N_CORES):
        rows = np.concatenate(
            [np.arange(g * R + k * RC, g * R + (k + 1) * RC) for g in range(4)])
        wk = np.concatenate(
            [Wi0[:, rows, :].transpose(0, 2, 1),
             Wh0[:, rows, :].transpose(0, 2, 1),
             Wh1[:, rows, :].transpose(0, 2, 1),
             Wi1[:, rows, :].transpose(0, 2, 1)], axis=1)
        wall = np.ascontiguousarray(
            wk.reshape(U_RUN, NK0 + NK1, 128, W).transpose(0, 2, 1, 3)
            .reshape(U_RUN, 128, (NK0 + NK1) * W)).astype(fp8)
        in_maps.append({
            "xT": xT,
            "h0T0": h0T,
            "h1T0": h1T,
            "c0i": np.ascontiguousarray(c0i[:, k * RC : (k + 1) * RC]),
            "c1i": np.ascontiguousarray(c1i[:, k * RC : (k + 1) * RC]),
            "wall": wall,
            "b0": np.ascontiguousarray(b0_full[:, rows].reshape(1, -1)).astype(bf),
            "b1": np.ascontiguousarray(b1_full[:, rows].reshape(1, -1)).astype(bf),
        })
    return in_maps


def assemble_output(inputs: dict, results: list[dict]) -> np.ndarray:
    st = np.asarray(inputs["init_states_input"], np.float32).reshape(B, 2 * L, R)
    h1i = st[:, 2]
    out = np.empty((B, U * R), np.float32)
    out[:, :R] = h1i
    for k in range(N_CORES):
        y = results[k]["y"]
        for s in range(U_RUN):
            out[:, (s + 1) * R + k * RC : (s + 1) * R + (k + 1) * RC] = y[s]
    return out


PHASE_MARKS: list = []


def _mark(nc, label):
    PHASE_MARKS.append((label, nc.next_id()))


_CACHE: dict = {}


def _get_compiled():
    if "nc" not in _CACHE:
        _CACHE["nc"] = build_program(reps=1)
    return _CACHE["nc"]


def kernel(**inputs) -> np.ndarray:
    from concourse.bass_utils import run_bass_kernel_spmd

    nc = _get_compiled()
    in_maps = prepare_in_maps(inputs)
    res = run_bass_kernel_spmd(nc, in_maps, list(range(N_CORES)))
    return assemble_output(inputs, res.results)


# revision 5
# speedup vs baseline: 2.3729x; 2.3729x over previous
"""Trainium2 Bass kernel v3 — fp8 weights + software-pipelined schedule.

Differences vs v2:
- Reordered steady-state iteration so the PE queue never head-blocks:
  A=pre1wh(t), C=pre1wi(t), D=pre0h(t+1), tg1 transposes, B=pre0x(t+2)
  (PE filler during the gate0/AG window), tg0 transposes. The layer-0
  pipeline runs one step ahead of layer 1, so both AllGathers launch evenly
  spaced and their lands complete before the consuming matmuls.
- h lands split into 4 chunked DMAs (2 k-tiles each) so pre1wh can start
  ~1us after the collective completes.
- ps0 double-buffered (pre0x for step t+2 is issued while gate0 of step
  t+1 still reads the previous buffer).
"""
import numpy as np

B, I, R, L, U = 256, 512, 1024, 2, 11
U_RUN = U - 1
N_CORES = 8
RC = 128
W = 4 * RC
NKX = I // 128         # 4
NKH = R // 128         # 8
NK0 = NKX + NKH        # 12
NK1 = 2 * NKH          # 16
NB = B // 128
BH = 128
WSCALE = 64.0


def build_program(reps: int = 1):
    import concourse.bacc as bacc
    import concourse.mybir as mybir
    import concourse.tile as tile
    import concourse.masks as masks

    F32 = mybir.dt.float32
    BF16 = mybir.dt.bfloat16
    FP8 = mybir.dt.float8e4
    Sig = mybir.ActivationFunctionType.Sigmoid
    Tanh = mybir.ActivationFunctionType.Tanh
    INV = 1.0 / WSCALE

    PHASE_MARKS.clear()
    nc = bacc.Bacc("TRN2", target_bir_lowering=False, debug=False,
                   num_devices=N_CORES)

    xT_d = nc.dram_tensor("xT", [I, B], BF16, kind="ExternalInput")
    h0T_d = nc.dram_tensor("h0T0", [R, B], FP8, kind="ExternalInput")
    h1T_d = nc.dram_tensor("h1T0", [R, B], BF16, kind="ExternalInput")
    c0_d = nc.dram_tensor("c0i", [B, RC], F32, kind="ExternalInput")
    c1_d = nc.dram_tensor("c1i", [B, RC], F32, kind="ExternalInput")
    wall_d = nc.dram_tensor("wall", [U_RUN, 128, (NK0 + NK1) * W], FP8,
                            kind="ExternalInput")
    b0_d = nc.dram_tensor("b0", [1, U_RUN * W], BF16, kind="ExternalInput")
    b1_d = nc.dram_tensor("b1", [1, U_RUN * W], BF16, kind="ExternalInput")
    y_d = nc.dram_tensor("y", [U_RUN, B, RC], BF16, kind="ExternalOutput")

    with tile.TileContext(nc) as tc:
        with tc.tile_pool(name="const", bufs=1) as constp, \
             tc.tile_pool(name="hpool", bufs=2) as hpool, \
             tc.tile_pool(name="cpool", bufs=2) as cpool, \
             tc.tile_pool(name="wpool", bufs=3) as wpool, \
             tc.tile_pool(name="gpool", bufs=2) as gpool, \
             tc.tile_pool(name="stage", bufs=2) as stage, \
             tc.tile_pool(name="ps0", bufs=2, space="PSUM") as ps0, \
             tc.tile_pool(name="ps1", bufs=1, space="PSUM") as ps1, \
             tc.tile_pool(name="psT", bufs=2, space="PSUM") as psT, \
             tc.tile_pool(name="dramp", bufs=2, space="DRAM") as dramp:

            ident = constp.tile([128, 128], BF16)
            masks.make_identity(nc, ident[:])
            ones = constp.tile([1, 128], BF16)
            nc.gpsimd.memset(ones[:], 1.0)
            xT_tiles = []
            for kk in range(NKX):
                xt = constp.tile([128, B], BF16, name=f"xT_{kk}")
                nc.sync.dma_start(xt[:], xT_d.ap()[kk * 128 : (kk + 1) * 128, :])
                xT_tiles.append(xt)
            bias0 = constp.tile([1, U_RUN * W], BF16)
            nc.sync.dma_start(bias0[:], b0_d.ap())
            bias1 = constp.tile([1, U_RUN * W], BF16)
            nc.sync.dma_start(bias1[:], b1_d.ap())

            def load_h_half(layer, b, src_ap, chunks=2):
                """[R, BH] h^T half -> [128, NKH*BH] tile (fp8 for L0)."""
                hdt = FP8 if layer == 0 else BF16
                t = hpool.tile([128, NKH * BH], hdt, name=f"h{layer}T_{b}",
                               tag=f"h{layer}T_{b}")
                step = NKH // chunks
                for ci in range(chunks):
                    lo = ci * step
                    nc.sync.dma_start(
                        t[:, lo * BH : (lo + step) * BH]
                        .rearrange("p (k b) -> p k b", k=step),
                        src_ap[lo * 128 : (lo + step) * 128, :]
                        .rearrange("(k p) b -> p k b", p=128),
                    )
                return t

            def load_wall(src_ap):
                ta = wpool.tile([128, NK0 * W], FP8, name="w0_s", tag="w0_s")
                for lo in range(0, NK0, 4):
                    hi = min(lo + 4, NK0)
                    nc.scalar.dma_start(ta[:, lo * W : hi * W],
                                        src_ap[:, lo * W : hi * W])
                tb = wpool.tile([128, NK1 * W], FP8, name="w1_s", tag="w1_s")
                for lo in range(0, NK1, 4):
                    hi = min(lo + 4, NK1)
                    nc.scalar.dma_start(tb[:, lo * W : hi * W],
                                        src_ap[:, (NK0 + lo) * W : (NK0 + hi) * W])
                return ta, tb

            def gate_cell(layer, b, pre_ps, c_old, hn_tile):
                sig = gpool.tile([128, 3 * RC], F32, name=f"sig{layer}_{b}",
                                 tag=f"sig{layer}_{b}")
                nc.scalar.activation(sig[:], pre_ps[:, : 3 * RC], Sig, scale=INV)
                tg = gpool.tile([128, RC], F32, name=f"tg{layer}_{b}",
                                tag=f"tg{layer}_{b}")
                nc.scalar.activation(tg[:], pre_ps[:, 3 * RC :], Tanh, scale=INV)
                t1 = gpool.tile([128, RC], F32, name=f"t1{layer}_{b}",
                                tag=f"t1{layer}_{b}")
                nc.vector.tensor_mul(t1[:], sig[:, :RC], tg[:])
                t2 = gpool.tile([128, RC], F32, name=f"t2{layer}_{b}",
                                tag=f"t2{layer}_{b}")
                nc.vector.tensor_mul(t2[:], sig[:, RC : 2 * RC], c_old[:])
                cnew = cpool.tile([128, RC], F32, name=f"c{layer}_{b}",
                                  tag=f"c{layer}_{b}")
                nc.vector.tensor_add(cnew[:], t1[:], t2[:])
                tcc = gpool.tile([128, RC], F32, name=f"tc{layer}_{b}",
                                 tag=f"tc{layer}_{b}")
                nc.scalar.activation(tcc[:], cnew[:], Tanh)
                nc.vector.tensor_mul(hn_tile[:, b * RC : (b + 1) * RC],
                                     sig[:, 2 * RC : 3 * RC], tcc[:])
                return cnew

            def gather_half(layer, b, hn_tile):
                """Transpose + cast + stage + AllGather for one half.
                Layer 0 ships fp8 (h0 tolerates it; h1 does not)."""
                hdt = FP8 if layer == 0 else BF16
                tps = psT.tile([128, BH], BF16, name=f"tps{layer}{b}", tag="tps")
                nc.tensor.transpose(tps[:], hn_tile[:, b * RC : (b + 1) * RC],
                                    ident[:])
                hst = stage.tile([128, BH], hdt, name=f"hst{layer}_{b}",
                                 tag=f"hst{layer}_{b}")
                nc.vector.tensor_copy(hst[:], tps[:])
                ag_in = dramp.tile([128, BH], hdt, name=f"agin{layer}_{b}",
                                   tag=f"agin{layer}_{b}")
                nc.sync.dma_start(ag_in[:], hst[:])
                ag_out = dramp.tile([R, BH], hdt, name=f"agout{layer}_{b}",
                                    tag=f"agout{layer}_{b}",
                                    addr_space="Shared")
                nc.gpsimd.collective_compute(
                    "AllGather", mybir.AluOpType.bypass,
                    replica_groups=[list(range(N_CORES))],
                    ins=[ag_in[:]], outs=[ag_out[:]],
                )
                return ag_out

            def hs(h_t, kk):
                return h_t[:, kk * BH : (kk + 1) * BH]

            def mm(p, lhsT, wslice, start, stop):
                nc.tensor.matmul(p[:], lhsT, wslice, start=start, stop=stop)

            def pre0_start(widx, wall_t, b):
                """bias + x part for step widx (opens the ps0 group)."""
                w0_s, _ = wall_t
                p = ps0.tile([128, W], F32, name=f"pre0_{b}", tag=f"pre0_{b}")
                mm(p, ones[:], bias0[:, widx * W : (widx + 1) * W], True, False)
                for kk in range(NKX):
                    mm(p, xT_tiles[kk][:, b * BH : (b + 1) * BH],
                       w0_s[:, kk * W : (kk + 1) * W], False, False)
                return p

            def pre0_finish(p, h0T_b, wall_t):
                w0_s, _ = wall_t
                for j in range(NKH // 2):
                    nc.tensor.matmul(
                        p[:],
                        h0T_b[:, 2 * j * BH : (2 * j + 2) * BH]
                        .rearrange("p (k b) -> p k b", k=2),
                        w0_s[:, (NKX + 2 * j) * W : (NKX + 2 * j + 2) * W]
                        .rearrange("p (k w) -> p k w", k=2),
                        start=False, stop=j == NKH // 2 - 1,
                        perf_mode=mybir.MatmulPerfMode.DoubleRow)

            def pre1_wh(widx, wall_t, h1T_b, b):
                _, w1_s = wall_t
                p = ps1.tile([128, W], F32, name=f"pre1_{b}", tag=f"pre1_{b}")
                mm(p, ones[:], bias1[:, widx * W : (widx + 1) * W], True, False)
                for kh in range(NKH):
                    mm(p, hs(h1T_b, kh),
                       w1_s[:, kh * W : (kh + 1) * W], False, False)
                return p

            def pre1_wi(p, h0T_b, wall_t):
                _, w1_s = wall_t
                for j in range(NKH // 2):
                    nc.tensor.matmul(
                        p[:],
                        h0T_b[:, 2 * j * BH : (2 * j + 2) * BH]
                        .rearrange("p (k b) -> p k b", k=2),
                        w1_s[:, (NKH + 2 * j) * W : (NKH + 2 * j + 2) * W]
                        .rearrange("p (k w) -> p k w", k=2),
                        start=False, stop=j == NKH // 2 - 1,
                        perf_mode=mybir.MatmulPerfMode.DoubleRow)

            # ---- prologue ----
            h0T = {b: load_h_half(0, b, h0T_d.ap()[:, b * BH:(b + 1) * BH])
                   for b in range(NB)}
            h1T = {b: load_h_half(1, b, h1T_d.ap()[:, b * BH:(b + 1) * BH])
                   for b in range(NB)}
            c = {}
            for layer, cd in ((0, c0_d), (1, c1_d)):
                for b in range(NB):
                    t = cpool.tile([128, RC], F32, name=f"c{layer}_{b}",
                                   tag=f"c{layer}_{b}")
                    nc.sync.dma_start(t[:], cd.ap()[b * 128 : (b + 1) * 128, :])
                    c[layer, b] = t

            wall = {0: load_wall(wall_d.ap()[0]), 1: load_wall(wall_d.ap()[1])}
            # step-0 layer0 fully in prologue
            pre0_pend = [pre0_start(0, wall[0], b) for b in range(NB)]
            for b in range(NB):
                pre0_finish(pre0_pend[b], h0T[b], wall[0])
            h0new = gpool.tile([128, NB * RC], BF16, name="h0new", tag="h0new")
            for b in range(NB):
                c[0, b] = gate_cell(0, b, pre0_pend[b], c[0, b], h0new)
                ag = gather_half(0, b, h0new)
                h0T[b] = load_h_half(0, b, ag[:])
            # open step-1 layer0 group (bias + x part) as PE filler
            pre0_pend = [pre0_start(1, wall[1], b) for b in range(NB)]

            # ---- steady-state iterations ----
            for rep in range(reps):
                for t in range(U_RUN):
                    has_next = not (rep == reps - 1 and t == U_RUN - 1)
                    write_y = rep == 0
                    nidx = (t + 1) % U_RUN
                    n2idx = (t + 2) % U_RUN

                    if has_next:
                        _mark(nc, f"r{rep}t{t}:wdma")
                        wall[n2idx] = load_wall(wall_d.ap()[n2idx])

                    _mark(nc, f"r{rep}t{t}:pre1")
                    wt = wall[t % U_RUN]
                    pre1 = []
                    for b in range(NB):
                        p = pre1_wh(t, wt, h1T[b], b)
                        pre1_wi(p, h0T[b], wt)
                        pre1.append(p)

                    if has_next:
                        _mark(nc, f"r{rep}t{t}:pre0h")
                        for b in range(NB):
                            pre0_finish(pre0_pend[b], h0T[b], wall[nidx])

                    _mark(nc, f"r{rep}t{t}:gate1")
                    h1new = gpool.tile([128, NB * RC], BF16, name="h1new",
                                       tag="h1new")
                    ag1 = {}
                    for b in range(NB):
                        c[1, b] = gate_cell(1, b, pre1[b], c[1, b], h1new)
                        if has_next:
                            ag1[b] = gather_half(1, b, h1new)
                    if write_y:
                        nc.sync.dma_start(
                            y_d.ap()[t].rearrange("(nb p) r -> p nb r", p=128),
                            h1new[:].rearrange("p (nb r) -> p nb r", nb=NB))

                    if has_next:
                        _mark(nc, f"r{rep}t{t}:gate0")
                        h0new = gpool.tile([128, NB * RC], BF16, name="h0new",
                                           tag="h0new")
                        ag0 = {}
                        for b in range(NB):
                            c[0, b] = gate_cell(0, b, pre0_pend[b], c[0, b],
                                                h0new)
                            ag0[b] = gather_half(0, b, h0new)

                        has_n2 = not (rep == reps - 1 and t == U_RUN - 2)
                        if has_n2:
                            _mark(nc, f"r{rep}t{t}:xpart")
                            pre0_pend = [pre0_start(n2idx, wall[n2idx], b)
                                         for b in range(NB)]

                        _mark(nc, f"r{rep}t{t}:lands")
                        for b in range(NB):
                            h1T[b] = load_h_half(1, b, ag1[b][:])
                        for b in range(NB):
                            h0T[b] = load_h_half(0, b, ag0[b][:])

    _mark(nc, "end")
    nc.compile()
    return nc


def prepare_in_maps(inputs: dict) -> list[dict]:
    import ml_dtypes
    bf = ml_dtypes.bfloat16
    fp8 = ml_dtypes.float8_e4m3

    x = np.ascontiguousarray(np.asarray(inputs["x"], np.float32))
    st = np.asarray(inputs["init_states_input"], np.float32).reshape(B, 2 * L, R)
    h0i, c0i, h1i, c1i = st[:, 0], st[:, 1], st[:, 2], st[:, 3]

    xT = x.T.astype(bf)
    h0T = h0i.T.astype(fp8)
    h1T = h1i.T.astype(bf)

    Wi0 = np.asarray(inputs["Wi0"], np.float32)[:U_RUN] * WSCALE
    Wh0 = np.asarray(inputs["Wh0"], np.float32)[:U_RUN] * WSCALE
    Wi1 = np.asarray(inputs["Wi1"], np.float32)[:U_RUN] * WSCALE
    Wh1 = np.asarray(inputs["Wh1"], np.float32)[:U_RUN] * WSCALE
    b0_full = (np.asarray(inputs["bi0"], np.float32)
               + np.asarray(inputs["bh0"], np.float32))[:U_RUN] * WSCALE
    b1_full = (np.asarray(inputs["bi1"], np.float32)
               + np.asarray(inputs["bh1"], np.float32))[:U_RUN] * WSCALE

    in_maps = []
    for k in range(N_CORES):
        rows = np.concatenate(
            [np.arange(g * R + k * RC, g * R + (k + 1) * RC) for g in range(4)])
        wk = np.concatenate(
            [Wi0[:, rows, :].transpose(0, 2, 1),
             Wh0[:, rows, :].transpose(0, 2, 1),
             Wh1[:, rows, :].transpose(0, 2, 1),
             Wi1[:, rows, :].transpose(0, 2, 1)], axis=1)
        wall = np.ascontiguousarray(
            wk.reshape(U_RUN, NK0 + NK1, 128, W).transpose(0, 2, 1, 3)
            .reshape(U_RUN, 128, (NK0 + NK1) * W)).astype(fp8)
        in_maps.append({
            "xT": xT,
            "h0T0": h0T,
            "h1T0": h1T,
            "c0i": np.ascontiguousarray(c0i[:, k * RC : (k + 1) * RC]),
            "c1i": np.ascontiguousarray(c1i[:, k * RC : (k + 1) * RC]),
            "wall": wall,
            "b0": np.ascontiguousarray(b0_full[:, rows].reshape(1, -1)).astype(bf),
            "b1": np.ascontiguousarray(b1_full[:, rows].reshape(1, -1)).astype(bf),
        })
    return in_maps


def assemble_output(inputs: dict, results: list[dict]) -> np.ndarray:
    st = np.asarray(inputs["init_states_input"], np.float32).reshape(B, 2 * L, R)
    h1i = st[:, 2]
    out = np.empty((B, U * R), np.float32)
    out[:, :R] = h1i
    for k in range(N_CORES):
        y = results[k]["y"]
        for s in range(U_RUN):
            out[:, (s + 1) * R + k * RC : (s + 1) * R + (k + 1) * RC] = y[s]
    return out


PHASE_MARKS: list = []


def _mark(nc, label):
    PHASE_MARKS.append((label, nc.next_id()))


_CACHE: dict = {}


def _get_compiled():
    if "nc" not in _CACHE:
        _CACHE["nc"] = build_program(reps=1)
    return _CACHE["nc"]


def kernel(**inputs) -> np.ndarray:
    from concourse.bass_utils import run_bass_kernel_spmd

    nc = _get_compiled()
    in_maps = prepare_in_maps(inputs)
    res = run_bass_kernel_spmd(nc, in_maps, list(range(N_CORES)))
    return assemble_output(inputs, res.results)
